# revision 18
# baseline (speedup 1.0000x reference)
"""Bahdanau-attention kernel for Trainium2, data-parallel over 8 NeuronCores.

V4: bf16 datapath, all tensors host-pre-transposed/laid-out so the device
does zero transposes and only a handful of coarse contiguous HWDGE DMAs.
GpSimd (Pool) is never used — its software sequencer adds ~2us per
semaphore op and poisons cross-engine dependency latency.

Per core (B_local=8, T=1024, H=1024), per batch b:
  eT[o,t]   = tanh(sum_h W_enc[o,h] x[t,h] + (W_dec h + b_dec + b_enc)[o])
              (PE bf16 matmuls from xT/wT; ScalarE tanh w/ per-partition bias)
  scores[t] = sum_o w_score[o] * eT[o,t]        (PE, ws chunk stationary)
  u         = exp(scores); w = u / sum(u)       (ScalarE exp+accum, DVE recip)
  u_rep     = ones^T w                          (PE K=1 matmul -> [128,T])
  ctxT[p,j] = sum_t xT[j][p,t] * u_rep[p,t]     (DVE fused scalar_tensor_tensor)
b_score dropped: softmax is shift-invariant so it cancels in both outputs.
Outputs: out_w [bl,T] rows; out_ctx [bl,P,NH] column-major (host transposes).
"""

import sys

if "/opt/trn_rl_repo" not in sys.path:
    sys.path.insert(0, "/opt/trn_rl_repo")

import numpy as np

B, T, H = 64, 1024, 1024
NCORES = 8
BL = B // NCORES  # batches per core
P = 128  # partitions
NH = H // P  # h chunks
NO = H // P  # o chunks
NS = 2  # free-dim halves of T
F = 512  # matmul free-dim slice (one PSUM bank of f32)

_CACHE = {}
LAST_RESULT = None


def build(bl=BL):
    import concourse.tile as tile
    from concourse import bacc, mybir

    f32 = mybir.dt.float32
    f32r = mybir.dt.float32r
    bf16 = mybir.dt.bfloat16
    AF = mybir.ActivationFunctionType
    OP = mybir.AluOpType

    nc = bacc.Bacc("TRN2", target_bir_lowering=False, debug=False, num_devices=NCORES)
    # host-prepared layouts:
    #   x_d[b, h, t]        (x transposed per batch)
    #   we_d[h, o], wd_d[h, o]  (W^T)
    #   hst_d[p, c*bl+b] = hidden_state[b, c*P+p]
    #   bsum_d[p, c] = (b_enc + b_dec)[c*P+p];  ws_d[p, c] = w_score[c*P+p]
    x_d = nc.declare_dram_parameter("spatial_feats", [bl, H, T], bf16, isOutput=False)
    hst_d = nc.declare_dram_parameter("hsT", [P, NH * bl], bf16, isOutput=False)
    we_d = nc.declare_dram_parameter("W_enc", [H, H], bf16, isOutput=False)
    wd_d = nc.declare_dram_parameter("W_dec", [H, H], bf16, isOutput=False)
    bsum_d = nc.declare_dram_parameter("bsum", [P, NH], f32, isOutput=False)
    ws_d = nc.declare_dram_parameter("w_score", [P, NH], bf16, isOutput=False)
    ctx_d = nc.declare_dram_parameter("out_ctx", [bl, P, NH], f32, isOutput=True)
    wout_d = nc.declare_dram_parameter("out_w", [bl, T], f32, isOutput=True)
    z_d = nc.declare_dram_parameter("out_z", [bl, 2], f32, isOutput=True)

    with tile.TileContext(nc) as tc:
        with (
            tc.tile_pool(name="const", bufs=1) as const,
            tc.tile_pool(name="wt", bufs=1) as wt_pool,
            tc.tile_pool(name="wdt", bufs=1) as wdt_pool,
            tc.tile_pool(name="xt", bufs=3) as xt_pool,
            tc.tile_pool(name="et", bufs=8) as et_pool,
            tc.tile_pool(name="urep", bufs=2) as urep_pool,
            tc.tile_pool(name="ctxsc", bufs=2) as ctxsc_pool,
            tc.tile_pool(name="ctxt", bufs=2) as ctxt_pool,
            tc.tile_pool(name="rows", bufs=4) as rows,
            tc.tile_pool(name="small", bufs=16) as small,
            tc.tile_pool(name="mmps", bufs=6, space="PSUM") as mm_ps,
            tc.tile_pool(name="scps", bufs=2, space="PSUM") as sc_ps,
        ):
            # ---- constants / small loads (scalar queue) ----
            ones_f = const.tile([1, P], f32, tag="onesf")
            nc.vector.memset(ones_f[:], 1.0)
            ones_col = const.tile([1, P], f32r, tag="ones")
            nc.vector.tensor_copy(ones_col[:], ones_f[:])

            bsum = const.tile([P, NH], f32, tag="bsum")
            ws_sb = const.tile([P, NH], bf16, tag="ws")
            hsT = const.tile([P, NH * bl], bf16, tag="hsT")

            # ---- W_enc^T and x(0)^T first (enc start gates everything);
            # W_dec^T afterwards on sync — its bias matmuls are emitted
            # mid-batch-0 so the PE never stalls on it ----
            wT = wt_pool.tile([P, NH * H], bf16, tag="wt")
            xT0 = xt_pool.tile([P, NH * T], bf16, tag="xt", name="xT0")
            # first half as two quarters so enc(0) starts as early as possible
            for j0, j1 in ((0, 2), (2, 4), (4, 8)):
                r0, r1 = j0 * P, j1 * P
                nc.sync.dma_start(
                    wT[:, j0 * H : j1 * H].rearrange("p (j o) -> p j o", j=j1 - j0),
                    we_d[r0:r1, :].rearrange("(j p) o -> p j o", p=P),
                )
                nc.scalar.dma_start(
                    xT0[:, j0 * T : j1 * T].rearrange("p (j t) -> p j t", j=j1 - j0),
                    x_d[0, r0:r1, :].rearrange("(j p) t -> p j t", p=P),
                )
            nc.scalar.dma_start(bsum[:], bsum_d[:])
            nc.scalar.dma_start(ws_sb[:], ws_d[:])
            nc.scalar.dma_start(hsT[:], hst_d[:])
            wdT = wdt_pool.tile([P, NH * H], bf16, tag="wdt")
            nc.sync.dma_start(
                wdT[:].rearrange("p (j o) -> p j o", j=NH),
                wd_d[:].rearrange("(j p) o -> p j o", p=P),
            )
            bias_all = const.tile([P, NO * bl], f32, tag="bias_all")

            def emit_bias():
                # bias_all[o_off, ot*bl+b] = (W_dec h_b + b_dec + b_enc)[o]
                # psd from ur_ps (idle during batch 0) to keep mm_ps free
                for ot in range(NO):
                    psd = mm_ps.tile([P, F], f32, tag="mmps", name=f"psd{ot}")
                    for j in range(NH):
                        nc.tensor.matmul(
                            psd[:, 0:bl],
                            wdT[:, j * H + ot * P : j * H + (ot + 1) * P],
                            hsT[:, j * bl : (j + 1) * bl],
                            start=(j == 0),
                            stop=(j == NH - 1),
                        )
                    nc.vector.tensor_scalar_add(
                        bias_all[:, ot * bl : (ot + 1) * bl],
                        psd[:, 0:bl],
                        bsum[:, ot : ot + 1],
                    )

            def emit_enc_mms(xT, o, s):
                ps = mm_ps.tile([P, F], f32, tag="mmps")
                for h in range(NH):
                    nc.tensor.matmul(
                        ps[:],
                        wT[:, h * H + o * P : h * H + (o + 1) * P],
                        xT[:, h * T + s * F : h * T + (s + 1) * F],
                        start=(h == 0),
                        stop=(h == NH - 1),
                    )
                return ps

            def emit_tanh(ps, it, o):
                e = et_pool.tile([P, F], bf16, tag="et")
                nc.scalar.activation(
                    e[:],
                    ps[:],
                    AF.Tanh,
                    bias=bias_all[:, o * bl + it : o * bl + it + 1],
                    scale=1.0,
                )
                return e

            def emit_finish(st):
                """Batch-tail work for batch st['it']: replicate normalized
                weights across partitions, fused multiply-reduce context,
                DMA both outputs."""
                it = st["it"]
                u_n = st["u_n"]
                urp = []
                for s in range(NS):
                    pu = mm_ps.tile([P, F], f32, tag="mmps")
                    nc.tensor.matmul(
                        pu[:],
                        ones_col[:],
                        u_n[0:1, s * F : (s + 1) * F],
                        start=True,
                        stop=True,
                    )
                    urp.append(pu)
                u_rep = urep_pool.tile([P, T], bf16, tag="urep")
                for s in range(NS):
                    nc.scalar.activation(
                        u_rep[:, s * F : (s + 1) * F], urp[s][:], AF.Copy
                    )
                ctxT = ctxt_pool.tile([P, NH], f32, tag="ctxt")
                for j in range(NH):
                    scr = ctxsc_pool.tile([P, T], bf16, tag="ctxsc")
                    nc.vector.scalar_tensor_tensor(
                        out=scr[:],
                        in0=st["xT"][:, j * T : (j + 1) * T],
                        scalar=1.0,
                        in1=u_rep[:],
                        op0=OP.mult,
                        op1=OP.mult,
                        accum_out=ctxT[:, j : j + 1],
                    )
                nc.sync.dma_start(ctx_d[it], ctxT[:])
                nc.sync.dma_start(wout_d[it : it + 1, :], u_n[:].bitcast(f32))
                nc.sync.dma_start(z_d[it : it + 1, :], st["zz"][:])

            # ---- main per-batch pipeline (software-pipelined across the
            # batch boundary) ----
            # scores lag enc by TWO o-groups; the last two scores groups and
            # the exp of batch it are emitted inside batch it+1's first two
            # enc slots, so the PE never waits on ScalarE's tanh at a batch
            # boundary. finish(it-1) (u-replicate + fused context + DMAs)
            # lands in slot o==2.
            prev = None   # finish state (awaiting u-replicate + context)
            carry = None  # scores o6/o7 + exp of the previous batch
            xT = xT0

            def emit_carry_scores(c, o):
                for s in range(NS):
                    nc.tensor.matmul(
                        c["pss"][s][:],
                        ws_sb[:, o : o + 1],
                        c["eT"][(o, s)][:],
                        start=False,
                        stop=(o == NO - 1),
                    )

            def emit_exp(c):
                u_row = rows.tile([1, T], f32r, tag="urow")
                zz = small.tile([1, 2], f32, tag="zz")
                for s in range(NS):
                    nc.scalar.activation(
                        u_row[0:1, s * F : (s + 1) * F],
                        c["pss"][s][:],
                        AF.Exp,
                        bias=0.0,
                        scale=1.0,
                        accum_out=zz[0:1, s : s + 1],
                    )
                return {"xT": c["xT"], "u_n": u_row, "it": c["it"], "zz": zz}

            for it in range(bl):
                eT = {}
                psq = {}
                pss = None
                xT_next = None
                tanh_lag = 1 if it == 0 else 0
                for o in range(NO):
                    for s in range(NS):
                        psq[(o, s)] = emit_enc_mms(xT, o, s)
                    if o == 0 and it + 1 < bl:
                        xT_next = xt_pool.tile(
                            [P, NH * T], bf16, tag="xt", name=f"xT{it + 1}"
                        )
                        nc.sync.dma_start(
                            xT_next[:].rearrange("p (j t) -> p j t", j=NH),
                            x_d[it + 1].rearrange("(j p) t -> p j t", p=P),
                        )
                    if it == 0 and o == 1:
                        emit_bias()
                    if o >= tanh_lag:
                        oo = o - tanh_lag
                        for s in range(NS):
                            eT[(oo, s)] = emit_tanh(psq.pop((oo, s)), it, oo)
                    if carry is not None:
                        if o == 0:
                            emit_carry_scores(carry, NO - 2)
                        elif o == 1:
                            emit_carry_scores(carry, NO - 1)
                            prev = emit_exp(carry)
                            carry = None
                    if o == 2:
                        if prev is not None:
                            emit_finish(prev)
                            prev = None
                        pss = {
                            s: sc_ps.tile([1, F], f32, tag="scps", name=f"pss{s}")
                            for s in range(NS)
                        }
                    if o >= 2:
                        oo = o - 2
                        for s in range(NS):
                            nc.tensor.matmul(
                                pss[s][:],
                                ws_sb[:, oo : oo + 1],
                                eT[(oo, s)][:],
                                start=(oo == 0),
                                stop=False,
                            )
                if tanh_lag:
                    for s in range(NS):
                        eT[(NO - 1, s)] = emit_tanh(psq.pop((NO - 1, s)), it, NO - 1)
                carry = {"xT": xT, "eT": eT, "pss": pss, "it": it}
                if xT_next is not None:
                    xT = xT_next
            # drain: last batch's scores o6/o7 + exp + finish
            emit_carry_scores(carry, NO - 2)
            emit_carry_scores(carry, NO - 1)
            prev = emit_exp(carry)
            emit_finish(prev)

    nc.compile()
    return nc


def _get_nc(bl=BL):
    if bl not in _CACHE:
        _CACHE[bl] = build(bl)
    return _CACHE[bl]


def kernel(**inputs):
    import ml_dtypes
    from concourse.bass_utils import run_bass_kernel_spmd

    bf = ml_dtypes.bfloat16
    # host-side marshaling: bf16 conversion + all layout prep (transposes,
    # bias/score/hidden-state relayouts) so the device does zero transposes
    x = np.ascontiguousarray(
        np.asarray(inputs["spatial_feats"], dtype=np.float32)
        .astype(bf)
        .transpose(0, 2, 1)
    )
    hs = np.asarray(inputs["hidden_state"], dtype=np.float32).astype(bf)
    bsum = (
        np.asarray(inputs["b_enc"], dtype=np.float32)
        + np.asarray(inputs["b_dec"], dtype=np.float32)
    ).reshape(NH, P).T
    shared = {
        "W_enc": np.ascontiguousarray(
            np.asarray(inputs["W_enc"], dtype=np.float32).astype(bf).T
        ),
        "W_dec": np.ascontiguousarray(
            np.asarray(inputs["W_dec"], dtype=np.float32).astype(bf).T
        ),
        "bsum": np.ascontiguousarray(bsum),
        "w_score": np.ascontiguousarray(
            np.asarray(inputs["w_score"], dtype=np.float32).astype(bf).reshape(NH, P).T
        ),
    }
    nc = _get_nc()
    in_maps = []
    for i in range(NCORES):
        hs_slice = hs[i * BL : (i + 1) * BL]
        hsT = np.ascontiguousarray(
            hs_slice.reshape(BL, NH, P).transpose(2, 1, 0).reshape(P, NH * BL)
        )
        m = {
            "spatial_feats": x[i * BL : (i + 1) * BL],
            "hsT": hsT,
        }
        m.update(shared)
        in_maps.append(m)
    res = run_bass_kernel_spmd(nc, in_maps, core_ids=list(range(NCORES)))
    global LAST_RESULT
    LAST_RESULT = res
    # device outputs are unnormalized (u = exp(scores), ctx_raw = x^T u);
    # divide by z = sum(u) here
    ctx = np.concatenate(
        [
            res.results[i]["out_ctx"].transpose(0, 2, 1).reshape(BL, H)
            for i in range(NCORES)
        ],
        axis=0,
    )
    u = np.concatenate([res.results[i]["out_w"] for i in range(NCORES)], axis=0)
    z = np.concatenate([res.results[i]["out_z"] for i in range(NCORES)], axis=0)
    zs = z.sum(axis=1, dtype=np.float64).astype(np.float32)[:, None]
    return (ctx / zs, u / zs)


# revision 19
# speedup vs baseline: 1.0026x; 1.0026x over previous
"""Bahdanau-attention kernel for Trainium2, data-parallel over 8 NeuronCores.

V4: bf16 datapath, all tensors host-pre-transposed/laid-out so the device
does zero transposes and only a handful of coarse contiguous HWDGE DMAs.
GpSimd (Pool) is never used — its software sequencer adds ~2us per
semaphore op and poisons cross-engine dependency latency.

Per core (B_local=8, T=1024, H=1024), per batch b:
  eT[o,t]   = tanh(sum_h W_enc[o,h] x[t,h] + (W_dec h + b_dec + b_enc)[o])
              (PE bf16 matmuls from xT/wT; ScalarE tanh w/ per-partition bias)
  scores[t] = sum_o w_score[o] * eT[o,t]        (PE, ws chunk stationary)
  u         = exp(scores); w = u / sum(u)       (ScalarE exp+accum, DVE recip)
  u_rep     = ones^T w                          (PE K=1 matmul -> [128,T])
  ctxT[p,j] = sum_t xT[j][p,t] * u_rep[p,t]     (DVE fused scalar_tensor_tensor)
b_score dropped: softmax is shift-invariant so it cancels in both outputs.
Outputs: out_w [bl,T] rows; out_ctx [bl,P,NH] column-major (host transposes).
"""

import sys

if "/opt/trn_rl_repo" not in sys.path:
    sys.path.insert(0, "/opt/trn_rl_repo")

import numpy as np

B, T, H = 64, 1024, 1024
NCORES = 8
BL = B // NCORES  # batches per core
P = 128  # partitions
NH = H // P  # h chunks
NO = H // P  # o chunks
NS = 2  # free-dim halves of T
F = 512  # matmul free-dim slice (one PSUM bank of f32)

_CACHE = {}
LAST_RESULT = None


def build(bl=BL):
    import concourse.tile as tile
    from concourse import bacc, mybir

    f32 = mybir.dt.float32
    f32r = mybir.dt.float32r
    bf16 = mybir.dt.bfloat16
    AF = mybir.ActivationFunctionType
    OP = mybir.AluOpType

    nc = bacc.Bacc("TRN2", target_bir_lowering=False, debug=False, num_devices=NCORES)
    # host-prepared layouts:
    #   x_d[b, h, t]        (x transposed per batch)
    #   we_d[h, o], wd_d[h, o]  (W^T)
    #   hst_d[p, c*bl+b] = hidden_state[b, c*P+p]
    #   bsum_d[p, c] = (b_enc + b_dec)[c*P+p];  ws_d[p, c] = w_score[c*P+p]
    x_d = nc.declare_dram_parameter("spatial_feats", [bl, H, T], bf16, isOutput=False)
    hst_d = nc.declare_dram_parameter("hsT", [P, NH * bl], bf16, isOutput=False)
    we_d = nc.declare_dram_parameter("W_enc", [H, H], bf16, isOutput=False)
    wd_d = nc.declare_dram_parameter("W_dec", [H, H], bf16, isOutput=False)
    bsum_d = nc.declare_dram_parameter("bsum", [P, NH], f32, isOutput=False)
    ws_d = nc.declare_dram_parameter("w_score", [P, NH], bf16, isOutput=False)
    ctx_d = nc.declare_dram_parameter("out_ctx", [bl, P, NH], f32, isOutput=True)
    wout_d = nc.declare_dram_parameter("out_w", [bl, T], f32, isOutput=True)
    z_d = nc.declare_dram_parameter("out_z", [bl, 2], f32, isOutput=True)

    with tile.TileContext(nc) as tc:
        with (
            tc.tile_pool(name="const", bufs=1) as const,
            tc.tile_pool(name="wt", bufs=1) as wt_pool,
            tc.tile_pool(name="wdt", bufs=1) as wdt_pool,
            tc.tile_pool(name="xt", bufs=3) as xt_pool,
            tc.tile_pool(name="et", bufs=6) as et_pool,
            tc.tile_pool(name="urep", bufs=2) as urep_pool,
            tc.tile_pool(name="ctxsc", bufs=2) as ctxsc_pool,
            tc.tile_pool(name="ctxt", bufs=2) as ctxt_pool,
            tc.tile_pool(name="rows", bufs=4) as rows,
            tc.tile_pool(name="small", bufs=16) as small,
            tc.tile_pool(name="mmps", bufs=4, space="PSUM") as mm_ps,
            tc.tile_pool(name="scps", bufs=2, space="PSUM") as sc_ps,
            tc.tile_pool(name="urps", bufs=2, space="PSUM") as ur_ps,
        ):
            # ---- constants / small loads (scalar queue) ----
            ones_f = const.tile([1, P], f32, tag="onesf")
            nc.vector.memset(ones_f[:], 1.0)
            ones_col = const.tile([1, P], f32r, tag="ones")
            nc.vector.tensor_copy(ones_col[:], ones_f[:])

            bsum = const.tile([P, NH], f32, tag="bsum")
            ws_sb = const.tile([P, NH], bf16, tag="ws")
            hsT = const.tile([P, NH * bl], bf16, tag="hsT")

            # ---- W_enc^T and x(0)^T first (enc start gates everything);
            # W_dec^T afterwards on sync — its bias matmuls are emitted
            # mid-batch-0 so the PE never stalls on it ----
            wT = wt_pool.tile([P, NH * H], bf16, tag="wt")
            xT0 = xt_pool.tile([P, NH * T], bf16, tag="xt", name="xT0")
            # first half as two quarters so enc(0) starts as early as possible
            for j0, j1 in ((0, 2), (2, 4), (4, 8)):
                r0, r1 = j0 * P, j1 * P
                nc.sync.dma_start(
                    wT[:, j0 * H : j1 * H].rearrange("p (j o) -> p j o", j=j1 - j0),
                    we_d[r0:r1, :].rearrange("(j p) o -> p j o", p=P),
                )
                nc.scalar.dma_start(
                    xT0[:, j0 * T : j1 * T].rearrange("p (j t) -> p j t", j=j1 - j0),
                    x_d[0, r0:r1, :].rearrange("(j p) t -> p j t", p=P),
                )
            nc.scalar.dma_start(bsum[:], bsum_d[:])
            nc.scalar.dma_start(ws_sb[:], ws_d[:])
            nc.scalar.dma_start(hsT[:], hst_d[:])
            wdT = wdt_pool.tile([P, NH * H], bf16, tag="wdt")
            nc.sync.dma_start(
                wdT[:].rearrange("p (j o) -> p j o", j=NH),
                wd_d[:].rearrange("(j p) o -> p j o", p=P),
            )
            bias_all = const.tile([P, NO * bl], f32, tag="bias_all")

            def emit_bias():
                # bias_all[o_off, ot*bl+b] = (W_dec h_b + b_dec + b_enc)[o]
                # psd from ur_ps (idle during batch 0) to keep mm_ps free
                for ot in range(NO):
                    psd = ur_ps.tile([P, F], f32, tag="urps", name=f"psd{ot}")
                    for j in range(NH):
                        nc.tensor.matmul(
                            psd[:, 0:bl],
                            wdT[:, j * H + ot * P : j * H + (ot + 1) * P],
                            hsT[:, j * bl : (j + 1) * bl],
                            start=(j == 0),
                            stop=(j == NH - 1),
                        )
                    nc.vector.tensor_scalar_add(
                        bias_all[:, ot * bl : (ot + 1) * bl],
                        psd[:, 0:bl],
                        bsum[:, ot : ot + 1],
                    )

            def emit_enc_mms(xT, o, s):
                ps = mm_ps.tile([P, F], f32, tag="mmps")
                for h in range(NH):
                    nc.tensor.matmul(
                        ps[:],
                        wT[:, h * H + o * P : h * H + (o + 1) * P],
                        xT[:, h * T + s * F : h * T + (s + 1) * F],
                        start=(h == 0),
                        stop=(h == NH - 1),
                    )
                return ps

            def emit_tanh(ps, it, o):
                e = et_pool.tile([P, F], bf16, tag="et")
                nc.scalar.activation(
                    e[:],
                    ps[:],
                    AF.Tanh,
                    bias=bias_all[:, o * bl + it : o * bl + it + 1],
                    scale=1.0,
                )
                return e

            def emit_finish(st):
                """Batch-tail work for batch st['it']: replicate normalized
                weights across partitions, fused multiply-reduce context,
                DMA both outputs."""
                it = st["it"]
                u_n = st["u_n"]
                urp = []
                for s in range(NS):
                    pu = ur_ps.tile([P, F], f32, tag="urps")
                    nc.tensor.matmul(
                        pu[:],
                        ones_col[:],
                        u_n[0:1, s * F : (s + 1) * F],
                        start=True,
                        stop=True,
                    )
                    urp.append(pu)
                u_rep = urep_pool.tile([P, T], bf16, tag="urep")
                for s in range(NS):
                    nc.scalar.activation(
                        u_rep[:, s * F : (s + 1) * F], urp[s][:], AF.Copy
                    )
                ctxT = ctxt_pool.tile([P, NH], f32, tag="ctxt")
                for j in range(NH):
                    scr = ctxsc_pool.tile([P, T], bf16, tag="ctxsc")
                    nc.vector.scalar_tensor_tensor(
                        out=scr[:],
                        in0=st["xT"][:, j * T : (j + 1) * T],
                        scalar=1.0,
                        in1=u_rep[:],
                        op0=OP.mult,
                        op1=OP.mult,
                        accum_out=ctxT[:, j : j + 1],
                    )
                nc.sync.dma_start(ctx_d[it], ctxT[:])
                nc.sync.dma_start(wout_d[it : it + 1, :], u_n[:].bitcast(f32))
                nc.sync.dma_start(z_d[it : it + 1, :], st["zz"][:])

            # ---- main per-batch pipeline (software-pipelined across the
            # batch boundary) ----
            # scores lag enc by TWO o-groups; the last two scores groups and
            # the exp of batch it are emitted inside batch it+1's first two
            # enc slots, so the PE never waits on ScalarE's tanh at a batch
            # boundary. finish(it-1) (u-replicate + fused context + DMAs)
            # lands in slot o==2.
            prev = None   # finish state (awaiting u-replicate + context)
            carry = None  # scores o6/o7 + exp of the previous batch
            xT = xT0

            def emit_carry_scores(c, o):
                for s in range(NS):
                    nc.tensor.matmul(
                        c["pss"][s][:],
                        ws_sb[:, o : o + 1],
                        c["eT"][(o, s)][:],
                        start=False,
                        stop=(o == NO - 1),
                    )

            def emit_exp(c):
                u_row = rows.tile([1, T], f32r, tag="urow")
                zz = small.tile([1, 2], f32, tag="zz")
                for s in range(NS):
                    nc.scalar.activation(
                        u_row[0:1, s * F : (s + 1) * F],
                        c["pss"][s][:],
                        AF.Exp,
                        bias=0.0,
                        scale=1.0,
                        accum_out=zz[0:1, s : s + 1],
                    )
                return {"xT": c["xT"], "u_n": u_row, "it": c["it"], "zz": zz}

            for it in range(bl):
                eT = {}
                psq = {}
                pss = None
                xT_next = None
                tanh_lag = 1 if it == 0 else 0
                for o in range(NO):
                    for s in range(NS):
                        psq[(o, s)] = emit_enc_mms(xT, o, s)
                    if o == 0 and it + 1 < bl:
                        xT_next = xt_pool.tile(
                            [P, NH * T], bf16, tag="xt", name=f"xT{it + 1}"
                        )
                        nc.sync.dma_start(
                            xT_next[:].rearrange("p (j t) -> p j t", j=NH),
                            x_d[it + 1].rearrange("(j p) t -> p j t", p=P),
                        )
                    if it == 0 and o == 1:
                        emit_bias()
                    if o >= tanh_lag:
                        oo = o - tanh_lag
                        for s in range(NS):
                            eT[(oo, s)] = emit_tanh(psq.pop((oo, s)), it, oo)
                    if carry is not None:
                        if o == 0:
                            emit_carry_scores(carry, NO - 2)
                        elif o == 1:
                            emit_carry_scores(carry, NO - 1)
                            prev = emit_exp(carry)
                            carry = None
                    if o == 2:
                        if prev is not None:
                            emit_finish(prev)
                            prev = None
                        pss = {
                            s: sc_ps.tile([1, F], f32, tag="scps", name=f"pss{s}")
                            for s in range(NS)
                        }
                    if o >= 2:
                        oo = o - 2
                        for s in range(NS):
                            nc.tensor.matmul(
                                pss[s][:],
                                ws_sb[:, oo : oo + 1],
                                eT[(oo, s)][:],
                                start=(oo == 0),
                                stop=False,
                            )
                if tanh_lag:
                    for s in range(NS):
                        eT[(NO - 1, s)] = emit_tanh(psq.pop((NO - 1, s)), it, NO - 1)
                carry = {"xT": xT, "eT": eT, "pss": pss, "it": it}
                if xT_next is not None:
                    xT = xT_next
            # drain: last batch's scores o6/o7 + exp + finish
            emit_carry_scores(carry, NO - 2)
            emit_carry_scores(carry, NO - 1)
            prev = emit_exp(carry)
            emit_finish(prev)

    nc.compile()
    return nc


def _get_nc(bl=BL):
    if bl not in _CACHE:
        _CACHE[bl] = build(bl)
    return _CACHE[bl]


def kernel(**inputs):
    import ml_dtypes
    from concourse.bass_utils import run_bass_kernel_spmd

    bf = ml_dtypes.bfloat16
    # host-side marshaling: bf16 conversion + all layout prep (transposes,
    # bias/score/hidden-state relayouts) so the device does zero transposes
    x = np.ascontiguousarray(
        np.asarray(inputs["spatial_feats"], dtype=np.float32)
        .astype(bf)
        .transpose(0, 2, 1)
    )
    hs = np.asarray(inputs["hidden_state"], dtype=np.float32).astype(bf)
    bsum = (
        np.asarray(inputs["b_enc"], dtype=np.float32)
        + np.asarray(inputs["b_dec"], dtype=np.float32)
    ).reshape(NH, P).T
    shared = {
        "W_enc": np.ascontiguousarray(
            np.asarray(inputs["W_enc"], dtype=np.float32).astype(bf).T
        ),
        "W_dec": np.ascontiguousarray(
            np.asarray(inputs["W_dec"], dtype=np.float32).astype(bf).T
        ),
        "bsum": np.ascontiguousarray(bsum),
        "w_score": np.ascontiguousarray(
            np.asarray(inputs["w_score"], dtype=np.float32).astype(bf).reshape(NH, P).T
        ),
    }
    nc = _get_nc()
    in_maps = []
    for i in range(NCORES):
        hs_slice = hs[i * BL : (i + 1) * BL]
        hsT = np.ascontiguousarray(
            hs_slice.reshape(BL, NH, P).transpose(2, 1, 0).reshape(P, NH * BL)
        )
        m = {
            "spatial_feats": x[i * BL : (i + 1) * BL],
            "hsT": hsT,
        }
        m.update(shared)
        in_maps.append(m)
    res = run_bass_kernel_spmd(nc, in_maps, core_ids=list(range(NCORES)))
    global LAST_RESULT
    LAST_RESULT = res
    # device outputs are unnormalized (u = exp(scores), ctx_raw = x^T u);
    # divide by z = sum(u) here
    ctx = np.concatenate(
        [
            res.results[i]["out_ctx"].transpose(0, 2, 1).reshape(BL, H)
            for i in range(NCORES)
        ],
        axis=0,
    )
    u = np.concatenate([res.results[i]["out_w"] for i in range(NCORES)], axis=0)
    z = np.concatenate([res.results[i]["out_z"] for i in range(NCORES)], axis=0)
    zs = z.sum(axis=1, dtype=np.float64).astype(np.float32)[:, None]
    return (ctx / zs, u / zs)


# revision 20
# speedup vs baseline: 1.0065x; 1.0040x over previous
"""Bahdanau-attention kernel for Trainium2, data-parallel over 8 NeuronCores.

V4: bf16 datapath, all tensors host-pre-transposed/laid-out so the device
does zero transposes and only a handful of coarse contiguous HWDGE DMAs.
GpSimd (Pool) is never used — its software sequencer adds ~2us per
semaphore op and poisons cross-engine dependency latency.

Per core (B_local=8, T=1024, H=1024), per batch b:
  eT[o,t]   = tanh(sum_h W_enc[o,h] x[t,h] + (W_dec h + b_dec + b_enc)[o])
              (PE bf16 matmuls from xT/wT; ScalarE tanh w/ per-partition bias)
  scores[t] = sum_o w_score[o] * eT[o,t]        (PE, ws chunk stationary)
  u         = exp(scores); w = u / sum(u)       (ScalarE exp+accum, DVE recip)
  u_rep     = ones^T w                          (PE K=1 matmul -> [128,T])
  ctxT[p,j] = sum_t xT[j][p,t] * u_rep[p,t]     (DVE fused scalar_tensor_tensor)
b_score dropped: softmax is shift-invariant so it cancels in both outputs.
Outputs: out_w [bl,T] rows; out_ctx [bl,P,NH] column-major (host transposes).
"""

import sys

if "/opt/trn_rl_repo" not in sys.path:
    sys.path.insert(0, "/opt/trn_rl_repo")

import numpy as np

B, T, H = 64, 1024, 1024
NCORES = 8
BL = B // NCORES  # batches per core
P = 128  # partitions
NH = H // P  # h chunks
NO = H // P  # o chunks
NS = 2  # free-dim halves of T
F = 512  # matmul free-dim slice (one PSUM bank of f32)

_CACHE = {}
LAST_RESULT = None


def build(bl=BL):
    import concourse.tile as tile
    from concourse import bacc, mybir

    f32 = mybir.dt.float32
    f32r = mybir.dt.float32r
    bf16 = mybir.dt.bfloat16
    AF = mybir.ActivationFunctionType
    OP = mybir.AluOpType

    nc = bacc.Bacc("TRN2", target_bir_lowering=False, debug=False, num_devices=NCORES)
    # host-prepared layouts:
    #   x_d[b, h, t]        (x transposed per batch)
    #   we_d[h, o], wd_d[h, o]  (W^T)
    #   hst_d[p, c*bl+b] = hidden_state[b, c*P+p]
    #   bsum_d[p, c] = (b_enc + b_dec)[c*P+p];  ws_d[p, c] = w_score[c*P+p]
    x_d = nc.declare_dram_parameter("spatial_feats", [bl, H, T], bf16, isOutput=False)
    hst_d = nc.declare_dram_parameter("hsT", [P, NH * bl], bf16, isOutput=False)
    we_d = nc.declare_dram_parameter("W_enc", [H, H], bf16, isOutput=False)
    wd_d = nc.declare_dram_parameter("W_dec", [H, H], bf16, isOutput=False)
    bsum_d = nc.declare_dram_parameter("bsum", [P, NH], f32, isOutput=False)
    ws_d = nc.declare_dram_parameter("w_score", [P, NH], bf16, isOutput=False)
    ctx_d = nc.declare_dram_parameter("out_ctx", [bl, P, NH], f32, isOutput=True)
    wout_d = nc.declare_dram_parameter("out_w", [bl, T], f32, isOutput=True)
    z_d = nc.declare_dram_parameter("out_z", [bl, 2], f32, isOutput=True)

    with tile.TileContext(nc) as tc:
        with (
            tc.tile_pool(name="const", bufs=1) as const,
            tc.tile_pool(name="wt", bufs=1) as wt_pool,
            tc.tile_pool(name="wdt", bufs=1) as wdt_pool,
            tc.tile_pool(name="xt", bufs=3) as xt_pool,
            tc.tile_pool(name="et", bufs=6) as et_pool,
            tc.tile_pool(name="urep", bufs=2) as urep_pool,
            tc.tile_pool(name="ctxsc", bufs=2) as ctxsc_pool,
            tc.tile_pool(name="ctxt", bufs=2) as ctxt_pool,
            tc.tile_pool(name="rows", bufs=4) as rows,
            tc.tile_pool(name="small", bufs=16) as small,
            tc.tile_pool(name="mmps", bufs=4, space="PSUM") as mm_ps,
            tc.tile_pool(name="scps", bufs=2, space="PSUM") as sc_ps,
            tc.tile_pool(name="urps", bufs=2, space="PSUM") as ur_ps,
        ):
            # ---- constants / small loads (scalar queue) ----
            ones_f = const.tile([1, P], f32, tag="onesf")
            nc.vector.memset(ones_f[:], 1.0)
            ones_col = const.tile([1, P], f32r, tag="ones")
            nc.vector.tensor_copy(ones_col[:], ones_f[:])

            bsum = const.tile([P, NH], f32, tag="bsum")
            ws_sb = const.tile([P, NH], bf16, tag="ws")
            hsT = const.tile([P, NH * bl], bf16, tag="hsT")

            # ---- W_enc^T and x(0)^T first (enc start gates everything);
            # W_dec^T afterwards on sync — its bias matmuls are emitted
            # mid-batch-0 so the PE never stalls on it ----
            wT = wt_pool.tile([P, NH * H], bf16, tag="wt")
            xT0 = xt_pool.tile([P, NH * T], bf16, tag="xt", name="xT0")
            # first half as two quarters so enc(0) starts as early as possible
            for j0, j1 in ((0, 1), (1, 2), (2, 4), (4, 8)):
                r0, r1 = j0 * P, j1 * P
                nc.sync.dma_start(
                    wT[:, j0 * H : j1 * H].rearrange("p (j o) -> p j o", j=j1 - j0),
                    we_d[r0:r1, :].rearrange("(j p) o -> p j o", p=P),
                )
                nc.scalar.dma_start(
                    xT0[:, j0 * T : j1 * T].rearrange("p (j t) -> p j t", j=j1 - j0),
                    x_d[0, r0:r1, :].rearrange("(j p) t -> p j t", p=P),
                )
            nc.scalar.dma_start(bsum[:], bsum_d[:])
            nc.scalar.dma_start(ws_sb[:], ws_d[:])
            nc.scalar.dma_start(hsT[:], hst_d[:])
            wdT = wdt_pool.tile([P, NH * H], bf16, tag="wdt")
            nc.sync.dma_start(
                wdT[:].rearrange("p (j o) -> p j o", j=NH),
                wd_d[:].rearrange("(j p) o -> p j o", p=P),
            )
            bias_all = const.tile([P, NO * bl], f32, tag="bias_all")

            def emit_bias():
                # bias_all[o_off, ot*bl+b] = (W_dec h_b + b_dec + b_enc)[o]
                # psd from ur_ps (idle during batch 0) to keep mm_ps free
                for ot in range(NO):
                    psd = ur_ps.tile([P, F], f32, tag="urps", name=f"psd{ot}")
                    for j in range(NH):
                        nc.tensor.matmul(
                            psd[:, 0:bl],
                            wdT[:, j * H + ot * P : j * H + (ot + 1) * P],
                            hsT[:, j * bl : (j + 1) * bl],
                            start=(j == 0),
                            stop=(j == NH - 1),
                        )
                    nc.vector.tensor_scalar_add(
                        bias_all[:, ot * bl : (ot + 1) * bl],
                        psd[:, 0:bl],
                        bsum[:, ot : ot + 1],
                    )

            def emit_enc_mms(xT, o, s):
                ps = mm_ps.tile([P, F], f32, tag="mmps")
                for h in range(NH):
                    nc.tensor.matmul(
                        ps[:],
                        wT[:, h * H + o * P : h * H + (o + 1) * P],
                        xT[:, h * T + s * F : h * T + (s + 1) * F],
                        start=(h == 0),
                        stop=(h == NH - 1),
                    )
                return ps

            def emit_tanh(ps, it, o):
                e = et_pool.tile([P, F], bf16, tag="et")
                for hh in range(2):
                    nc.scalar.activation(
                        e[:, hh * (F // 2) : (hh + 1) * (F // 2)],
                        ps[:, hh * (F // 2) : (hh + 1) * (F // 2)],
                        AF.Tanh,
                        bias=bias_all[:, o * bl + it : o * bl + it + 1],
                        scale=1.0,
                    )
                return e

            def emit_finish(st, last=False):
                """Batch-tail work for batch st['it']: replicate raw
                exp-weights across partitions, fused multiply-reduce context,
                DMA the outputs. In `last` mode (pipeline drain, all engines
                otherwise idle) the context chunks are split DVE/ScalarE to
                shorten the serial tail."""
                it = st["it"]
                u_n = st["u_n"]
                urp = []
                for s in range(NS):
                    pu = ur_ps.tile([P, F], f32, tag="urps")
                    nc.tensor.matmul(
                        pu[:],
                        ones_col[:],
                        u_n[0:1, s * F : (s + 1) * F],
                        start=True,
                        stop=True,
                    )
                    urp.append(pu)
                u_rep = urep_pool.tile([P, T], bf16, tag="urep")
                for s in range(NS):
                    nc.scalar.activation(
                        u_rep[:, s * F : (s + 1) * F], urp[s][:], AF.Copy
                    )
                ctxT = ctxt_pool.tile([P, NH], f32, tag="ctxt")
                ndve = NH - 3 if last else NH
                for j in range(ndve):
                    scr = ctxsc_pool.tile([P, T], bf16, tag="ctxsc")
                    nc.vector.scalar_tensor_tensor(
                        out=scr[:],
                        in0=st["xT"][:, j * T : (j + 1) * T],
                        scalar=1.0,
                        in1=u_rep[:],
                        op0=OP.mult,
                        op1=OP.mult,
                        accum_out=ctxT[:, j : j + 1],
                    )
                for j in range(ndve, NH):
                    scr = ctxsc_pool.tile([P, T], bf16, tag="ctxsc")
                    nc.vector.tensor_mul(
                        scr[:], st["xT"][:, j * T : (j + 1) * T], u_rep[:]
                    )
                    nc.scalar.activation(
                        scr[:],
                        scr[:],
                        AF.Copy,
                        accum_out=ctxT[:, j : j + 1],
                    )
                nc.sync.dma_start(ctx_d[it], ctxT[:])
                nc.sync.dma_start(wout_d[it : it + 1, :], u_n[:].bitcast(f32))
                nc.sync.dma_start(z_d[it : it + 1, :], st["zz"][:])

            # ---- main per-batch pipeline (software-pipelined across the
            # batch boundary) ----
            # scores lag enc by TWO o-groups; the last two scores groups and
            # the exp of batch it are emitted inside batch it+1's first two
            # enc slots, so the PE never waits on ScalarE's tanh at a batch
            # boundary. finish(it-1) (u-replicate + fused context + DMAs)
            # lands in slot o==2.
            prev = None   # finish state (awaiting u-replicate + context)
            carry = None  # scores o6/o7 + exp of the previous batch
            xT = xT0

            def emit_carry_scores(c, o):
                for s in range(NS):
                    nc.tensor.matmul(
                        c["pss"][s][:],
                        ws_sb[:, o : o + 1],
                        c["eT"][(o, s)][:],
                        start=False,
                        stop=(o == NO - 1),
                    )

            def emit_exp(c):
                u_row = rows.tile([1, T], f32r, tag="urow")
                zz = small.tile([1, 2], f32, tag="zz")
                for s in range(NS):
                    nc.scalar.activation(
                        u_row[0:1, s * F : (s + 1) * F],
                        c["pss"][s][:],
                        AF.Exp,
                        bias=0.0,
                        scale=1.0,
                        accum_out=zz[0:1, s : s + 1],
                    )
                return {"xT": c["xT"], "u_n": u_row, "it": c["it"], "zz": zz}

            for it in range(bl):
                eT = {}
                psq = {}
                pss = None
                xT_next = None
                tanh_lag = 1 if it == 0 else 0
                for o in range(NO):
                    for s in range(NS):
                        psq[(o, s)] = emit_enc_mms(xT, o, s)
                    if o == 0 and it + 1 < bl:
                        xT_next = xt_pool.tile(
                            [P, NH * T], bf16, tag="xt", name=f"xT{it + 1}"
                        )
                        nc.sync.dma_start(
                            xT_next[:].rearrange("p (j t) -> p j t", j=NH),
                            x_d[it + 1].rearrange("(j p) t -> p j t", p=P),
                        )
                    if it == 0 and o == 1:
                        emit_bias()
                    if o >= tanh_lag:
                        oo = o - tanh_lag
                        for s in range(NS):
                            eT[(oo, s)] = emit_tanh(psq.pop((oo, s)), it, oo)
                    if carry is not None:
                        if o == 0:
                            emit_carry_scores(carry, NO - 2)
                        elif o == 1:
                            emit_carry_scores(carry, NO - 1)
                            prev = emit_exp(carry)
                            carry = None
                    if o == 2:
                        if prev is not None:
                            emit_finish(prev)
                            prev = None
                        pss = {
                            s: sc_ps.tile([1, F], f32, tag="scps", name=f"pss{s}")
                            for s in range(NS)
                        }
                    if o >= 2:
                        oo = o - 2
                        for s in range(NS):
                            nc.tensor.matmul(
                                pss[s][:],
                                ws_sb[:, oo : oo + 1],
                                eT[(oo, s)][:],
                                start=(oo == 0),
                                stop=False,
                            )
                if tanh_lag:
                    for s in range(NS):
                        eT[(NO - 1, s)] = emit_tanh(psq.pop((NO - 1, s)), it, NO - 1)
                carry = {"xT": xT, "eT": eT, "pss": pss, "it": it}
                if xT_next is not None:
                    xT = xT_next
            # drain: last batch's scores o6/o7 + exp + finish
            emit_carry_scores(carry, NO - 2)
            emit_carry_scores(carry, NO - 1)
            prev = emit_exp(carry)
            emit_finish(prev, last=True)

    nc.compile()
    return nc


def _get_nc(bl=BL):
    if bl not in _CACHE:
        _CACHE[bl] = build(bl)
    return _CACHE[bl]


def kernel(**inputs):
    import ml_dtypes
    from concourse.bass_utils import run_bass_kernel_spmd

    bf = ml_dtypes.bfloat16
    # host-side marshaling: bf16 conversion + all layout prep (transposes,
    # bias/score/hidden-state relayouts) so the device does zero transposes
    x = np.ascontiguousarray(
        np.asarray(inputs["spatial_feats"], dtype=np.float32)
        .astype(bf)
        .transpose(0, 2, 1)
    )
    hs = np.asarray(inputs["hidden_state"], dtype=np.float32).astype(bf)
    bsum = (
        np.asarray(inputs["b_enc"], dtype=np.float32)
        + np.asarray(inputs["b_dec"], dtype=np.float32)
    ).reshape(NH, P).T
    shared = {
        "W_enc": np.ascontiguousarray(
            np.asarray(inputs["W_enc"], dtype=np.float32).astype(bf).T
        ),
        "W_dec": np.ascontiguousarray(
            np.asarray(inputs["W_dec"], dtype=np.float32).astype(bf).T
        ),
        "bsum": np.ascontiguousarray(bsum),
        "w_score": np.ascontiguousarray(
            np.asarray(inputs["w_score"], dtype=np.float32).astype(bf).reshape(NH, P).T
        ),
    }
    nc = _get_nc()
    in_maps = []
    for i in range(NCORES):
        hs_slice = hs[i * BL : (i + 1) * BL]
        hsT = np.ascontiguousarray(
            hs_slice.reshape(BL, NH, P).transpose(2, 1, 0).reshape(P, NH * BL)
        )
        m = {
            "spatial_feats": x[i * BL : (i + 1) * BL],
            "hsT": hsT,
        }
        m.update(shared)
        in_maps.append(m)
    res = run_bass_kernel_spmd(nc, in_maps, core_ids=list(range(NCORES)))
    global LAST_RESULT
    LAST_RESULT = res
    # device outputs are unnormalized (u = exp(scores), ctx_raw = x^T u);
    # divide by z = sum(u) here
    ctx = np.concatenate(
        [
            res.results[i]["out_ctx"].transpose(0, 2, 1).reshape(BL, H)
            for i in range(NCORES)
        ],
        axis=0,
    )
    u = np.concatenate([res.results[i]["out_w"] for i in range(NCORES)], axis=0)
    z = np.concatenate([res.results[i]["out_z"] for i in range(NCORES)], axis=0)
    zs = z.sum(axis=1, dtype=np.float64).astype(np.float32)[:, None]
    return (ctx / zs, u / zs)
